# revision 22
# baseline (speedup 1.0000x reference)
"""Trainium2 Bass kernel: PhaseMultiHeadModel (complex phase attention + complex FF
+ ComplexNorm + vocab readout), SPMD over 8 NeuronCores with collectives.

v3 — chunked pipeline over 4 token chunks of 512 (2 per batch):
  * Head-parallel attention (core c owns heads 2c,2c+1) in bf16, with
    host-pretransposed embeddings (no PE transposes) and a direct
    v2h = zH^T @ mv matmul for the PV stationary operand.
  * Per-chunk AllGather #1 (bf16, 256KB/rank -> 2MB) of the post-attention
    state; per-chunk FF (output-dim parallel, bf16, full 2048-contraction);
    magnitude stats partial sums + tiny AllReduce (4KB); local ComplexNorm;
    per-chunk AllGather #2 of the normalized state.
  * Vocab-parallel readout (4000 cols/core) split into pass1 (chunk 0,
    starts ~80us in) + pass2 (chunks 1-3, vt-outer so weights stream once),
    bf16 weights/state, f32 psum, bf16 output (host upconverts).
"""

import math

import numpy as np

P = 128
NCORES = 8
B, S, V, D, H = 2, 1024, 32000, 1024, 16
HD = D // H
SB = B * S
NH = H // NCORES        # heads per core (2)
CS = 512                # token chunk size
NCH = SB // CS          # 4 chunks
KT = 16                 # 128-row blocks of the gathered (re,im) state
Vc = V // NCORES        # 4000
VCP = 4096              # padded vocab cols per core
NVT = VCP // P          # 32
EPS = 1.0e-5
RG = [list(range(NCORES))]


def build_nc():
    import concourse.bass as bass  # noqa: F401
    import concourse.mybir as mybir
    import concourse.tile as tile
    from concourse import bacc

    f32 = mybir.dt.float32
    f32r = mybir.dt.float32r
    bf16 = mybir.dt.bfloat16
    AF = mybir.ActivationFunctionType

    def r(ap):
        return ap.bitcast(f32r)

    nc = bacc.Bacc(num_devices=NCORES)

    mg2d = nc.dram_tensor("mg2", [P, B, NH, S], bf16, kind="ExternalInput")
    phcd = nc.dram_tensor("phc", [P, NH, S], bf16, kind="ExternalInput")
    mkd = nc.dram_tensor("mk", [NH, P, P], bf16, kind="ExternalInput")
    mvd = nc.dram_tensor("mv", [NH, P, P], bf16, kind="ExternalInput")
    stepd = nc.dram_tensor("stepm", [P, 1280], bf16, kind="ExternalInput")
    onesd = nc.dram_tensor("ones", [P, 2], f32, kind="ExternalInput")
    onesrd = nc.dram_tensor("onesr", [2, P], f32, kind="ExternalInput")
    ffAd = nc.dram_tensor("ffA", [KT, P, P], bf16, kind="ExternalInput")
    ffBd = nc.dram_tensor("ffB", [KT, P, P], bf16, kind="ExternalInput")
    w2d = nc.dram_tensor("w2t", [NVT, P, KT * P], bf16, kind="ExternalInput")
    outv = nc.dram_tensor("outv", [NVT, P, SB], bf16, kind="ExternalOutput")

    ag1_in = [
        nc.dram_tensor("ag1c0i", [NH, P, CS], bf16),
        nc.dram_tensor("ag1c1i", [NH, P, CS], bf16),
        nc.dram_tensor("ag1b1i", [NH, P, S], bf16),
    ]
    ag1_out = [
        nc.dram_tensor("ag1c0o", [KT, P, CS], bf16, addr_space="Shared"),
        nc.dram_tensor("ag1c1o", [KT, P, CS], bf16, addr_space="Shared"),
        nc.dram_tensor("ag1b1o", [KT, P, S], bf16, addr_space="Shared"),
    ]
    st_in = [nc.dram_tensor(f"st{c}i", [2, CS], f32) for c in range(NCH)]
    st_out = [nc.dram_tensor(f"st{c}o", [2, CS], f32) for c in range(NCH)]
    ag2_in = [nc.dram_tensor(f"ag2{c}i", [NH, P, CS], bf16) for c in range(NCH)]
    ag2_out = [
        nc.dram_tensor(f"ag2{c}o", [KT, P, CS], bf16, addr_space="Shared")
        for c in range(NCH)
    ]

    ctx_lp = nc.allow_low_precision(reason="bf16 compute is intentional")
    ctx_lp.__enter__()
    with tile.TileContext(nc) as tc:
        with (
            tc.tile_pool(name="const", bufs=1) as cpool,
            tc.tile_pool(name="state", bufs=1) as spool,
            tc.tile_pool(name="work", bufs=1) as wpool,
            tc.tile_pool(name="ep", bufs=3) as epool,
            tc.tile_pool(name="np_", bufs=2) as npool,
            tc.tile_pool(name="stp", bufs=3, space="PSUM") as stp,
            tc.tile_pool(name="pvp", bufs=2, space="PSUM") as pvp,
            tc.tile_pool(name="ffq", bufs=2, space="PSUM") as ffq,
            tc.tile_pool(name="smp", bufs=1, space="PSUM") as smp,
            tc.tile_pool(name="zfp", bufs=2) as zfp,
            tc.tile_pool(name="sqp", bufs=2) as sqp,
        ):
            # ---- constants (mg2 of batch 0 first: critical path) ----
            mgt = [wpool.tile([P, NH, S], bf16, tag=f"mgt{b}", name=f"mgt{b}") for b in range(B)]
            nc.sync.dma_start(mgt[0][:], mg2d[:, 0, :, :])
            phc_sb = cpool.tile([P, NH, S], bf16)
            nc.sync.dma_start(phc_sb[:], phcd[:])
            mk_sb = cpool.tile([P, NH, P], bf16)
            mv_sb = cpool.tile([P, NH, P], bf16)
            for j in range(NH):
                nc.sync.dma_start(mk_sb[:, j, :], mkd[j, :, :])
                nc.sync.dma_start(mv_sb[:, j, :], mvd[j, :, :])
            ones_col = cpool.tile([P, 1], f32r)
            nc.sync.dma_start(ones_col[:], r(onesd[:, 0:1]))
            step_sb = cpool.tile([P, 1280], bf16)
            nc.sync.dma_start(step_sb[:], stepd[:])
            faS = cpool.tile([P, KT, P], bf16)
            fbS = cpool.tile([P, KT, P], bf16)
            for kt in range(KT):
                nc.sync.dma_start(faS[:, kt, :], ffAd[kt, :, :])
                nc.sync.dma_start(fbS[:, kt, :], ffBd[kt, :, :])
            nc.sync.dma_start(mgt[1][:], mg2d[:, 1, :, :])

            zH = [spool.tile([P, NH, S], bf16, tag=f"zH{b}", name=f"zH{b}") for b in range(B)]
            k2h = [spool.tile([P, NH, S], bf16, tag=f"k2{b}", name=f"k2{b}") for b in range(B)]
            v2h = [
                spool.tile([P, NH, S // P, P], bf16, tag=f"v2{b}", name=f"v2{b}")
                for b in range(B)
            ]
            fre = [spool.tile([P, CS], f32, tag=f"fre{c}", name=f"fre{c}") for c in range(NCH)]
            fim = [spool.tile([P, CS], f32, tag=f"fim{c}", name=f"fim{c}") for c in range(NCH)]
            fmt = [spool.tile([P, CS], f32r, tag=f"fm{c}", name=f"fm{c}") for c in range(NCH)]

            if True:

                def prep(b):
                    """tanh(emb) * phase -> zH[b]; K/V rotations."""
                    magt = wpool.tile([P, NH, S], bf16, tag="magt")
                    for j in range(NH):
                        nc.scalar.activation(
                            magt[:, j, :], mgt[b][:, j, :], AF.Tanh
                        )
                        nc.vector.tensor_mul(
                            zH[b][:, j, :], magt[:, j, :], phc_sb[:, j, :]
                        )
                    for j in range(NH):
                        for hf in range(S // CS):
                            sl = slice(hf * CS, (hf + 1) * CS)
                            kps = stp.tile([P, CS], f32, tag="st")
                            nc.tensor.matmul(
                                kps[:], lhsT=mk_sb[:, j, :], rhs=zH[b][:, j, sl],
                                start=True, stop=True,
                            )
                            nc.scalar.copy(k2h[b][:, j, sl], kps[:])
                        for tb in range(S // P):
                            vps = stp.tile([P, CS], f32, tag="st")
                            nc.tensor.matmul(
                                vps[:, 0:P],
                                lhsT=zH[b][:, j, tb * P : (tb + 1) * P],
                                rhs=mv_sb[:, j, :],
                                start=True, stop=True,
                            )
                            nc.vector.tensor_copy(v2h[b][:, j, tb, :], vps[:, 0:P])

                def attn(c):
                    """causal phase attention for one 512-token chunk; adds
                    attention output into zH in place; ships AllGather #1."""
                    b, qc = c // 2, c % 2
                    q0 = qc * CS
                    ntt = (q0 + CS) // P
                    for j in range(NH):
                        pv = pvp.tile([P, CS], f32, tag="pv")
                        esum = sqp.tile([P, CS], f32r, tag="esum")
                        es = []
                        for tt in range(ntt):
                            t0 = tt * P
                            st = stp.tile([P, CS], f32, tag="st")
                            nc.tensor.matmul(
                                st[:],
                                lhsT=k2h[b][:, j, t0 : t0 + P],
                                rhs=zH[b][:, j, q0 : q0 + CS],
                                start=True, stop=True,
                            )
                            e = epool.tile([P, CS], bf16, tag="e")
                            nc.scalar.activation(e[:], st[:], AF.Exp)
                            if t0 + P - 1 > q0:
                                off = 640 + (q0 - t0)
                                nc.vector.tensor_mul(
                                    e[:], e[:], step_sb[:, off : off + CS]
                                )
                            nc.tensor.matmul(
                                pv[:],
                                lhsT=v2h[b][:, j, tt, :],
                                rhs=e[:],
                                start=(tt == 0),
                                stop=(tt == ntt - 1),
                            )
                            es.append(e)
                            if tt == 1:
                                nc.vector.tensor_add(esum[:], es[0][:], es[1][:])
                            elif tt > 1:
                                nc.vector.tensor_add(esum[:], esum[:], e[:])
                        sm = smp.tile([1, CS], f32, tag="sm")
                        nc.tensor.matmul(
                            sm[:], lhsT=ones_col[:], rhs=esum[:],
                            start=True, stop=True,
                        )
                        pvs = npool.tile([P, CS], f32, tag="pvs")
                        nc.vector.tensor_copy(pvs[:], pv[:])
                        rc = npool.tile([1, CS], f32, tag="rc")
                        nc.vector.reciprocal_approx_fast(rc[:], sm[:])
                        rc2 = npool.tile([1, CS], f32r, tag="rc2")
                        nc.vector.tensor_copy(rc2[:], rc[:])
                        rps = pvp.tile([P, CS], f32, tag="pv")
                        nc.tensor.matmul(
                            rps[:], lhsT=ones_row[:], rhs=rc2[:],
                            start=True, stop=True,
                        )
                        tmp = npool.tile([P, CS], bf16, tag="tmp")
                        nc.vector.tensor_mul(tmp[:], pvs[:], rps[:])
                        dst = slice(q0, q0 + CS)
                        nc.vector.tensor_add(
                            zH[b][:, j, dst], zH[b][:, j, dst], tmp[:]
                        )
                    if b == 0:
                        nc.sync.dma_start(
                            ag1_in[c][:, :, :].rearrange("j p t -> p j t"),
                            zH[b][:, :, q0 : q0 + CS],
                        )
                        nc.gpsimd.collective_compute(
                            "AllGather",
                            mybir.AluOpType.bypass,
                            replica_groups=RG,
                            ins=[ag1_in[c][:, :, :].opt()],
                            outs=[ag1_out[c][:, :, :].opt()],
                        )
                    else:
                        nc.sync.dma_start(
                            ag1_in[2][:, :, q0 : q0 + CS].rearrange(
                                "j p t -> p j t"
                            ),
                            zH[b][:, :, q0 : q0 + CS],
                        )
                        if qc == 1:
                            nc.gpsimd.collective_compute(
                                "AllGather",
                                mybir.AluOpType.bypass,
                                replica_groups=RG,
                                ins=[ag1_in[2][:, :, :].opt()],
                                outs=[ag1_out[2][:, :, :].opt()],
                            )

                def ffp(c):
                    """complex FF for my 128 output dims on one chunk, from the
                    gathered full state; magnitude + stats partials + AllReduce."""
                    b, qc = c // 2, c % 2
                    q0 = qc * CS
                    zf = zfp.tile([P, KT, CS], bf16, tag="zf")
                    if b == 0:
                        zsrc = ag1_out[c][:, :, :]
                    else:
                        zsrc = ag1_out[2][:, :, q0 : q0 + CS]
                    nc.scalar.dma_start(
                        zf[:, :, :], zsrc.rearrange("k p t -> p k t")
                    )
                    pre = ffq.tile([P, CS], f32, tag="ff")
                    pim = ffq.tile([P, CS], f32, tag="ff")
                    for kt in range(KT):
                        nc.tensor.matmul(
                            pre[:], lhsT=faS[:, kt, :], rhs=zf[:, kt, :],
                            start=(kt == 0), stop=(kt == KT - 1),
                        )
                        nc.tensor.matmul(
                            pim[:], lhsT=fbS[:, kt, :], rhs=zf[:, kt, :],
                            start=(kt == 0), stop=(kt == KT - 1),
                        )
                    nc.scalar.copy(fre[c][:], pre[:])
                    nc.vector.tensor_copy(fim[c][:], pim[:])
                    sq = sqp.tile([P, CS], f32r, tag="sq")
                    nc.vector.tensor_mul(sq[:], fre[c][:], fre[c][:])
                    sq2 = sqp.tile([P, CS], f32, tag="sq2")
                    nc.gpsimd.tensor_mul(sq2[:], fim[c][:], fim[c][:])
                    nc.vector.tensor_add(sq[:], sq[:], sq2[:])
                    nc.scalar.activation(fmt[c][:], sq[:], AF.Sqrt)
                    p1 = smp.tile([1, CS], f32, tag="sm")
                    nc.tensor.matmul(
                        p1[:], lhsT=ones_col[:], rhs=fmt[c][:],
                        start=True, stop=True,
                    )
                    s1 = npool.tile([1, CS], f32, tag="s1")
                    nc.vector.tensor_copy(s1[:], p1[:])
                    nc.sync.dma_start(st_in[c][0:1, :], s1[:])
                    p2 = smp.tile([1, CS], f32, tag="sm")
                    nc.tensor.matmul(
                        p2[:], lhsT=ones_col[:], rhs=sq[:],
                        start=True, stop=True,
                    )
                    s2 = npool.tile([1, CS], f32, tag="s2")
                    nc.vector.tensor_copy(s2[:], p2[:])
                    nc.sync.dma_start(st_in[c][1:2, :], s2[:])
                    nc.gpsimd.collective_compute(
                        "AllReduce",
                        mybir.AluOpType.add,
                        replica_groups=RG,
                        ins=[st_in[c][:, :].opt()],
                        outs=[st_out[c][:, :].opt()],
                    )

                def norm(c):
                    """ComplexNorm for one chunk from the reduced stats; ships
                    the normalized bf16 state via AllGather #2."""
                    ssum = npool.tile([1, CS], f32, tag="ssum")
                    nc.sync.dma_start(ssum[:], st_out[c][0:1, :])
                    ssq = npool.tile([1, CS], f32, tag="ssq")
                    nc.sync.dma_start(ssq[:], st_out[c][1:2, :])
                    mean = npool.tile([1, CS], f32, tag="mean")
                    nc.vector.tensor_scalar_mul(mean[:], ssum[:], 1.0 / D)
                    w1 = npool.tile([1, CS], f32, tag="w1")
                    nc.vector.tensor_mul(w1[:], mean[:], ssum[:])
                    nc.vector.tensor_sub(ssq[:], ssq[:], w1[:])
                    nc.vector.tensor_scalar_mul(ssq[:], ssq[:], 1.0 / (D - 1))
                    stdr = npool.tile([1, CS], f32, tag="stdr")
                    nc.scalar.activation(stdr[:], ssq[:], AF.Sqrt)
                    nc.vector.tensor_scalar_add(stdr[:], stdr[:], EPS)
                    rinv = npool.tile([1, CS], f32, tag="rinv")
                    nc.vector.reciprocal_approx_fast(rinv[:], stdr[:])
                    mrep = npool.tile([P, CS], f32, tag="mrep")
                    nc.gpsimd.partition_broadcast(mrep[:], mean[:])
                    rrep = npool.tile([P, CS], f32, tag="rrep")
                    nc.gpsimd.partition_broadcast(rrep[:], rinv[:])
                    xm = npool.tile([P, CS], f32, tag="xm")
                    nc.vector.tensor_sub(xm[:], fmt[c][:], mrep[:])
                    nc.vector.tensor_mul(xm[:], xm[:], rrep[:])
                    nc.scalar.activation(xm[:], xm[:], AF.Tanh)
                    rmt = npool.tile([P, CS], f32, tag="rmt")
                    nc.vector.tensor_scalar_add(rmt[:], fmt[c][:], EPS)
                    nc.vector.reciprocal_approx_fast(rmt[:], rmt[:])
                    nc.vector.tensor_mul(xm[:], xm[:], rmt[:])
                    zcre = npool.tile([P, CS], bf16, tag="zcre")
                    nc.vector.tensor_mul(zcre[:], fre[c][:], xm[:])
                    zcim = npool.tile([P, CS], bf16, tag="zcim")
                    nc.gpsimd.tensor_mul(zcim[:], fim[c][:], xm[:])
                    nc.sync.dma_start(ag2_in[c][0, :, :], zcre[:])
                    nc.sync.dma_start(ag2_in[c][1, :, :], zcim[:])
                    nc.gpsimd.collective_compute(
                        "AllGather",
                        mybir.AluOpType.bypass,
                        replica_groups=RG,
                        ins=[ag2_in[c][:, :, :].opt()],
                        outs=[ag2_out[c][:, :, :].opt()],
                    )

                prep(0)
                attn(0)
                attn(1)
                prep(1)
                attn(2)
                attn(3)
                with tc.tile_wait_until(0.058):
                    ffp(0)
                with tc.tile_wait_until(0.064):
                    ffp(1)
                with tc.tile_wait_until(0.080):
                    norm(0)
                with tc.tile_wait_until(0.086):
                    norm(1)
                with tc.tile_wait_until(0.100):
                    ffp(2)
                with tc.tile_wait_until(0.106):
                    ffp(3)
                norm(2)
                norm(3)

            # ---- vocab readout: pass1 = chunk 0, pass2 = chunks 1-3 ----
            with (
                tc.tile_pool(name="z2p", bufs=1) as z2p,
                tc.tile_pool(name="wvp", bufs=4) as wvp,
                tc.tile_pool(name="obp", bufs=6) as obp,
                tc.tile_pool(name="rops", bufs=8, space="PSUM") as rops,
            ):
                z2a = z2p.tile([P, KT, CS], bf16, tag="z2a")
                with tc.tile_wait_until(0.112):
                    nc.scalar.dma_start(
                        z2a[:, :, :],
                        ag2_out[0][:, :, :].rearrange("k p t -> p k t"),
                    )
                for vt in range(NVT):
                    wv = wvp.tile([P, KT * P], bf16, tag="w")
                    nc.scalar.dma_start(wv[:], w2d[vt, :, :])
                    ps = rops.tile([P, CS], f32, tag="ro")
                    for kb in range(KT):
                        nc.tensor.matmul(
                            ps[:],
                            lhsT=wv[:, kb * P : (kb + 1) * P],
                            rhs=z2a[:, kb, :],
                            start=(kb == 0),
                            stop=(kb == KT - 1),
                        )
                    ob = obp.tile([P, CS], bf16, tag="ob")
                    nc.vector.tensor_copy(ob[:], ps[:])
                    nc.gpsimd.dma_start(outv[vt, :, 0:CS], ob[:])

                z2b = z2p.tile([P, KT, NCH - 1, CS], bf16, tag="z2b")
                for kt in range(KT):
                    for ci in range(NCH - 1):
                        nc.sync.dma_start(
                            z2b[:, kt, ci, :], ag2_out[1 + ci][kt, :, :]
                        )
                for vt in range(NVT):
                    wv = wvp.tile([P, KT * P], bf16, tag="w")
                    nc.scalar.dma_start(wv[:], w2d[vt, :, :])
                    pss = [rops.tile([P, CS], f32, tag="ro", name=f"ro{_i}") for _i in range(NCH - 1)]
                    for kb in range(KT):
                        for ci in range(NCH - 1):
                            nc.tensor.matmul(
                                pss[ci][:],
                                lhsT=wv[:, kb * P : (kb + 1) * P],
                                rhs=z2b[:, kb, ci, :],
                                start=(kb == 0),
                                stop=(kb == KT - 1),
                            )
                    for ci in range(NCH - 1):
                        ob = obp.tile([P, CS], bf16, tag="ob")
                        if ci % 2 == 0:
                            nc.scalar.copy(ob[:], pss[ci][:])
                        else:
                            nc.vector.tensor_copy(ob[:], pss[ci][:])
                        nc.gpsimd.dma_start(
                            outv[vt, :, (1 + ci) * CS : (2 + ci) * CS], ob[:]
                        )

    ctx_lp.__exit__(None, None, None)
    nc.compile()
    return nc


def host_prep(x, emb, q_rot, k_rot, v_rot, ff_real, ff_imag, w_r, b_r, w_i, b_i):
    """Host-side sharding + constant table prep. Returns per-core input maps."""
    from ml_dtypes import bfloat16

    x = np.asarray(x)
    emb = np.asarray(emb, np.float32)
    q_rot = np.asarray(q_rot, np.float32)
    k_rot = np.asarray(k_rot, np.float32)
    v_rot = np.asarray(v_rot, np.float32)
    ff_real = np.asarray(ff_real, np.float32)
    ff_imag = np.asarray(ff_imag, np.float32)
    w_r = np.asarray(w_r, np.float32)
    w_i = np.asarray(w_i, np.float32)

    pos = np.arange(S, dtype=np.float32)[:, None]
    dim = np.arange(D, dtype=np.float32)[None, :]
    freq = np.exp(-(dim / D) * np.float32(math.log(10000.0)))
    ph = pos * freq * np.float32(math.pi)          # [S, D]
    cph_t = np.cos(ph).T.astype(np.float32)        # [D, S]
    sph_t = np.sin(ph).T.astype(np.float32)

    delta = q_rot - k_rot
    kc, ks = np.cos(delta), np.sin(delta)
    vcos, vsin = np.cos(v_rot), np.sin(v_rot)
    mk = np.zeros((H, 2 * HD, 2 * HD), np.float32)
    mv = np.zeros((H, 2 * HD, 2 * HD), np.float32)
    ar = np.arange(HD)
    for h in range(H):
        mk[h][ar, ar] = kc[h]
        mk[h][HD + ar, ar] = ks[h]
        mk[h][HD + ar, HD + ar] = kc[h]
        mk[h][ar, HD + ar] = -ks[h]
        mv[h][ar, ar] = vcos[h]
        mv[h][HD + ar, ar] = -vsin[h]
        mv[h][ar, HD + ar] = vsin[h]
        mv[h][HD + ar, HD + ar] = vcos[h]

    stepm = np.zeros((P, 1280), np.float32)
    ii = np.arange(1280)[None, :] - 640
    stepm[np.arange(P)[:, None] <= ii] = 1.0

    # ffA[h] = [ff_real rows h*64..; -ff_imag rows], ffB[h] = [ff_imag; ff_real]
    ffA = np.stack(
        [
            np.concatenate(
                [ff_real[h * HD : (h + 1) * HD, :], -ff_imag[h * HD : (h + 1) * HD, :]],
                axis=0,
            )
            for h in range(H)
        ]
    ).astype(np.float32)  # [16, 128, D]
    ffB = np.stack(
        [
            np.concatenate(
                [ff_imag[h * HD : (h + 1) * HD, :], ff_real[h * HD : (h + 1) * HD, :]],
                axis=0,
            )
            for h in range(H)
        ]
    ).astype(np.float32)

    mags = emb[x.reshape(-1)]  # [SB, D] host gather (index routing only)
    ones = np.ones((P, 2), np.float32)
    onesr = np.ones((2, P), np.float32)

    per_core = []
    for c in range(NCORES):
        cs = slice(P * c, P * (c + 1))
        # mg2[p, b, j, t]: pre-tanh magnitudes, re/im duplicated per head
        m_loc = mags[:, cs].reshape(B, S, NH, HD)      # [B, S, j, hd]
        mg2 = np.empty((P, B, NH, S), np.float32)
        for j in range(NH):
            blk = m_loc[:, :, j, :].transpose(2, 0, 1)  # [hd, B, S]
            mg2[0:HD, :, j, :] = blk
            mg2[HD:P, :, j, :] = blk
        # phc[p, j, t]: cos rows then sin rows of this core's dims
        phc = np.empty((P, NH, S), np.float32)
        for j in range(NH):
            gs = slice(P * c + j * HD, P * c + (j + 1) * HD)
            phc[0:HD, j, :] = cph_t[gs, :]
            phc[HD:P, j, :] = sph_t[gs, :]
        # vocab slice, padded to 4096
        wr = np.zeros((D, VCP), np.float32)
        wr[:, :Vc] = w_r[:, Vc * c : Vc * (c + 1)]
        wi = np.zeros((D, VCP), np.float32)
        wi[:, :Vc] = w_i[:, Vc * c : Vc * (c + 1)]
        wstack = np.empty((KT, P, VCP), np.float32)
        for rr in range(NCORES):
            wstack[2 * rr] = wr[P * rr : P * (rr + 1), :]
            wstack[2 * rr + 1] = wi[P * rr : P * (rr + 1), :]
        w2 = np.ascontiguousarray(
            wstack.reshape(KT, P, NVT, P).transpose(2, 1, 0, 3).reshape(NVT, P, KT * P)
        ).astype(bfloat16)

        per_core.append(
            dict(
                mg2=np.ascontiguousarray(mg2).astype(bfloat16),
                phc=np.ascontiguousarray(phc).astype(bfloat16),
                mk=np.ascontiguousarray(mk[2 * c : 2 * c + 2]).astype(bfloat16),
                mv=np.ascontiguousarray(mv[2 * c : 2 * c + 2]).astype(bfloat16),
                stepm=stepm.astype(bfloat16),
                ones=ones,
                onesr=onesr,
                ffA=np.ascontiguousarray(ffA[:, :, cs]).astype(bfloat16),
                ffB=np.ascontiguousarray(ffB[:, :, cs]).astype(bfloat16),
                w2t=w2,
            )
        )
    return per_core


_NC_CACHE = {}


def get_nc():
    if "nc" not in _NC_CACHE:
        _NC_CACHE["nc"] = build_nc()
    return _NC_CACHE["nc"]


def kernel(x, emb, q_rot, k_rot, v_rot, ff_real, ff_imag, w_r, b_r, w_i, b_i):
    from concourse.bass_utils import run_bass_kernel_spmd

    in_maps = host_prep(
        x, emb, q_rot, k_rot, v_rot, ff_real, ff_imag, w_r, b_r, w_i, b_i
    )
    nc = get_nc()
    res = run_bass_kernel_spmd(nc, in_maps, core_ids=list(range(NCORES)))
    # outv per core: [NVT, P, SB] vocab-major -> [SB, Vc] token-major slice
    chunks = [
        res.results[c]["outv"].astype(np.float32).reshape(VCP, SB)[:Vc, :].T
        for c in range(NCORES)
    ]
    logits = np.concatenate(chunks, axis=1).reshape(B, S, V)
    bias = np.asarray(b_r, np.float32) + np.asarray(b_i, np.float32)
    if bias.any():
        logits = logits + bias[None, None, :]
    return np.ascontiguousarray(logits.astype(np.float32))
